# revision 1
# baseline (speedup 1.0000x reference)
"""Trainium2 Bass kernel for the black-oil Peaceman loss (nn_Black_oil_peacemann).

Full inputs X:[4096,89,128] f32, Y:[4096,66,128] f32 -> out:[4096,66,128] f32.
Data-parallel over the batch axis: 512 samples per core on 8 cores; all math is
per-sample (the pressure mean is per-sample), the /N normalization uses the
global N=4096, so no cross-device communication is needed.

Per-core layout: samples on the 128 SBUF partitions (4 blocks of 128 samples),
channels*T on the free axis. Only the 67 used X channels are read from HBM
(perm 0:22, pressure 22, Sg 45:67, Sw 67:89); channels 23:45 are never loaded.

Algebra (constants folded; s = 1e-10/4096, K = 2*pi*DZ/ln(RE/RWELL)):
  p      = mean_t pressure;  dd = 100 - p;  m = min(p, 0.5)
  oil:   out = -s*Yo + (((Sg-0.7)*(0.8-Sw))^2 * ao) * perm,
         ao = CO*dd*exp(8e-5*m - 8e-6 - 1e-5*relu(p-0.5))
  water: out = -s*Yw + ((Sw-0.1)^2 * aw) * perm,  aw = CW*dd
  gas:   out = -s*Yg + (Sg^2 * ag) * perm,  ag = CG*dd/(mu_g(p)*bg(p))
Each phase is one ACT square/affine + two fused DVE scalar_tensor_tensor
passes (per-sample factor and perm-mult fused into one op; -s*Y + q fused
into the other, updating the Y tile in place so it doubles as the out tile).
Kernel is DMA-bound: ~123 us/engine of pure transfer at the ~26 GB/s
per-SDMA-engine fabric cap; DVE ~87 us and ACT ~50 us hide under it.
"""

import math
import sys

if "/opt/trn_rl_repo" not in sys.path:
    sys.path.insert(0, "/opt/trn_rl_repo")

import numpy as np

import concourse.bass as bass
import concourse.mybir as mybir
import concourse.tile as tile
from concourse.bass_utils import run_bass_kernel_spmd
from concourse.vector_clock import ScopedClock

F32 = mybir.dt.float32
AF = mybir.ActivationFunctionType
OP = mybir.AluOpType

N_CORES = 8
N_FULL = 4096
S_CORE = N_FULL // N_CORES  # 512 samples per core
BLK = 128                   # samples per block == SBUF partitions
N_BLK = S_CORE // BLK       # 4
T = 128
CW_CH = 22                  # wells per phase

S_NORM = np.float32(1e-10 / N_FULL)
RIGHT = float(np.log(np.float32(2.0)))       # ln(RE/RWELL), RE=400 RWELL=200
K_PEACE = 2.0 * math.pi * 100.0 / RIGHT      # 2*pi*DZ/right
C_W = float(np.float32(K_PEACE * (0.3 / 0.49) * float(S_NORM)))
C_G = float(np.float32(K_PEACE * (0.8 / 0.49) * float(S_NORM)))
C_O = float(np.float32(K_PEACE * (0.9 / 0.2401 / 2.5) * float(S_NORM)))

# bias constants shipped to SBUF via one DMA; order defines column index
_BIASES = [100.0, 0.0133, -1.7e-4, -0.5, -8e-6, 0.8, -0.1, 0.0]


def _patch_tile_drain():
    """walrus in this container rejects TPB_CTRL instructions carrying more
    than one sem wait ("Too many sync wait commands"); split the TileContext
    exit drain's waits into one-wait-per-instruction nops."""
    if getattr(tile.TileContext, "_drain_patched", False):
        return

    def _drain_and_barrier(self, tick_clock, wait_clock):
        nc = self.nc
        drain_inst = nc.sync.drain()
        wait_clock.add_sem_waits(
            drain_inst.ins, ScopedClock({None: tick_clock.global_clock})
        )
        si = drain_inst.ins.sync_info
        if si is not None and si.on_wait and len(si.on_wait) > 1:
            extra = list(si.on_wait[1:])
            del si.on_wait[1:]
            for w in extra:
                nop = nc.sync.nop(nofuse=True)
                nsi = nop.ins.sync_info
                if nsi is None:
                    nop.ins.sync_info = mybir.SyncInfo(on_wait=[w], on_update=[])
                else:
                    nsi.on_wait.append(w)

        nc.all_engine_barrier()
        assert self.sems is not None
        popped = nc._tile_sem_poison_stack.pop()
        assert popped is self._sem_poison
        nc.clear_and_free_semaphores(list(self.sems.allocated().values()))
        nc.all_engine_barrier()

    tile.TileContext._drain_and_barrier = _drain_and_barrier
    tile.TileContext._drain_patched = True


def _strip_init_barrier(nc):
    """Drop the Bass-init all-engine barrier (drain + EVSEM butterfly) from
    the entry block. Its EVSEM waits block every engine ~6.5us on runtime
    event-sem arming before the first DMA can issue. It only ordered the four
    init const memsets (t~0.3us, Pool) against their first compute reader
    (t~14us) -- a margin of ~14us makes the barrier unnecessary, and the
    kernel-tail barrier still runs long after arming completes."""
    bb = nc.m.functions[0].blocks[0]
    bb.instructions = [
        ins
        for ins in bb.instructions
        if type(ins).__name__ not in ("InstDrain", "InstEventSemaphore")
    ]


def _split_multi_waits(nc):
    """This container's walrus encodes at most one sem wait per instruction
    ("Too many sync wait commands"); hoist extra waits onto engine-matched
    nops inserted immediately before the offending instruction."""
    import bass_rust

    n = 0
    for f in nc.m.functions:
        for bb in f.blocks:
            out = []
            for ins in bb.instructions:
                si = ins.sync_info
                if si is not None and si.on_wait and len(si.on_wait) > 1:
                    keep = si.on_wait[-1]
                    for w in list(si.on_wait[:-1]):
                        nop = bass_rust.InstNoOp(
                            name=f"I-waitsplit-{n}", ins=[], outs=[]
                        )
                        n += 1
                        nop.engine = ins.engine
                        nop.sync_info = mybir.SyncInfo(on_wait=[w], on_update=[])
                        nc.register_instruction(nop)
                        out.append(nop)
                    del si.on_wait[:]
                    si.on_wait.append(keep)
                out.append(ins)
            bb.instructions = out


def _build():
    _patch_tile_drain()
    nc = bass.Bass(trn_type="TRN2")
    Xd = nc.dram_tensor("X", [S_CORE, 89, T], F32, kind="ExternalInput")
    Yd = nc.dram_tensor("Y", [S_CORE, 66, T], F32, kind="ExternalInput")
    Cd = nc.dram_tensor("C", [BLK, len(_BIASES)], F32, kind="ExternalInput")
    Od = nc.dram_tensor("O", [S_CORE, 66, T], F32, kind="ExternalOutput")

    with tile.TileContext(nc) as tc:
        with (
            tc.tile_pool(name="cst", bufs=1) as cst,
            tc.tile_pool(name="xa_p", bufs=3) as xap,
            tc.tile_pool(name="io", bufs=2) as iop,
            tc.tile_pool(name="tmp", bufs=3) as tp,
            tc.tile_pool(name="sc", bufs=2) as sp,
        ):
            cb = cst.tile([BLK, len(_BIASES)], F32)
            # issue on the store ring (ACT) so it doesn't delay block loads
            nc.scalar.dma_start(cb[:], Cd[:])

            def bias(idx):
                return cb[:, idx : idx + 1]

            for b in range(N_BLK):
                s0 = b * BLK
                s1 = s0 + BLK

                # channels 0:23 (perm + pressure) and 45:89 (Sg + Sw) are
                # contiguous in X -- one DMA each
                xa = xap.tile([BLK, 23, T], F32, tag="xa")
                nc.sync.dma_start(xa[:], Xd[s0:s1, 0:23, :])
                xb = iop.tile([BLK, 2 * CW_CH, T], F32, tag="xb")
                nc.sync.dma_start(xb[:], Xd[s0:s1, 45:89, :])
                y = iop.tile([BLK, 66, T], F32, tag="y")
                nc.sync.dma_start(y[:], Yd[s0:s1, :, :])
                perm = xa[:, 0:22, :]
                press = xa[:, 22:23, :]
                sg = xb[:, 0:22, :]
                sw = xb[:, 22:44, :]

                # ---- per-sample scalars ([128,1]) ----
                ps = sp.tile([BLK, 1], F32, tag="ps")
                nc.vector.reduce_sum(ps[:], press[:], axis=mybir.AxisListType.X)
                p = sp.tile([BLK, 1], F32, tag="p")
                nc.scalar.mul(p[:], ps[:], 1.0 / T)
                dd = sp.tile([BLK, 1], F32, tag="dd")
                nc.scalar.activation(
                    dd[:], p[:], AF.Identity, bias=bias(0), scale=-1.0
                )
                m = sp.tile([BLK, 1], F32, tag="m")
                nc.vector.tensor_scalar_min(m[:], p[:], 0.5)

                # oil factor ao = CO * dd * exp(8e-5*m - 8e-6 - 1e-5*relu(p-.5))
                r1 = sp.tile([BLK, 1], F32, tag="r1")
                nc.scalar.activation(r1[:], p[:], AF.Relu, bias=bias(3), scale=1.0)
                m8 = sp.tile([BLK, 1], F32, tag="m8")
                nc.scalar.activation(
                    m8[:], m[:], AF.Identity, bias=bias(4), scale=8e-5
                )
                tt = sp.tile([BLK, 1], F32, tag="tt")
                nc.vector.scalar_tensor_tensor(
                    tt[:], r1[:], -1e-5, m8[:], op0=OP.mult, op1=OP.add
                )
                ibo = sp.tile([BLK, 1], F32, tag="ibo")
                nc.scalar.activation(ibo[:], tt[:], AF.Exp)
                ao = sp.tile([BLK, 1], F32, tag="ao")
                nc.vector.scalar_tensor_tensor(
                    ao[:], ibo[:], C_O, dd[:], op0=OP.mult, op1=OP.mult
                )

                # water factor aw = CW * dd
                aw = sp.tile([BLK, 1], F32, tag="aw")
                nc.scalar.mul(aw[:], dd[:], C_W)

                # gas factor ag = CG * dd / (mu_g(p) * bg(p)); s2 = sqrt(ag)
                sqp = sp.tile([BLK, 1], F32, tag="sqp")
                nc.scalar.activation(sqp[:], p[:], AF.Square)
                pl = sp.tile([BLK, 1], F32, tag="pl")
                nc.scalar.activation(
                    pl[:], p[:], AF.Identity, bias=bias(1), scale=1e-6
                )
                mu = sp.tile([BLK, 1], F32, tag="mu")
                nc.vector.scalar_tensor_tensor(
                    mu[:], sqp[:], 3e-10, pl[:], op0=OP.mult, op1=OP.add
                )
                bgt = sp.tile([BLK, 1], F32, tag="bgt")
                nc.scalar.activation(
                    bgt[:], m[:], AF.Exp, bias=bias(2), scale=1.7e-3
                )
                den = sp.tile([BLK, 1], F32, tag="den")
                nc.vector.tensor_mul(den[:], mu[:], bgt[:])
                rg = sp.tile([BLK, 1], F32, tag="rg")
                nc.vector.reciprocal(rg[:], den[:])
                ag = sp.tile([BLK, 1], F32, tag="ag")
                nc.vector.scalar_tensor_tensor(
                    ag[:], rg[:], C_G, dd[:], op0=OP.mult, op1=OP.mult
                )

                # ---- main elementwise over [128, 22, 128] ----
                yo = y[:, 0:22, :]
                yw = y[:, 22:44, :]
                yg = y[:, 44:66, :]

                # oil (longest chain, DVE+ACT):
                a = tp.tile([BLK, CW_CH, T], F32, tag="tmp")
                nc.scalar.activation(
                    a[:], sw[:], AF.Identity, bias=bias(5), scale=-1.0
                )
                c = tp.tile([BLK, CW_CH, T], F32, tag="tmp")
                nc.vector.scalar_tensor_tensor(
                    c[:], sg[:], 0.7, a[:], op0=OP.subtract, op1=OP.mult
                )
                nc.scalar.activation(c[:], c[:], AF.Square)
                nc.vector.scalar_tensor_tensor(
                    c[:], c[:], ao[:], perm[:], op0=OP.mult, op1=OP.mult
                )
                nc.vector.scalar_tensor_tensor(
                    yo[:], yo[:], -float(S_NORM), c[:], op0=OP.mult, op1=OP.add
                )
                nc.scalar.dma_start(Od[s0:s1, 0:22, :], yo[:])

                # gas: yg = -s*Yg + (Sg^2 * ag) * perm
                u2 = tp.tile([BLK, CW_CH, T], F32, tag="tmp")
                nc.scalar.activation(u2[:], sg[:], AF.Square)
                nc.vector.scalar_tensor_tensor(
                    u2[:], u2[:], ag[:], perm[:], op0=OP.mult, op1=OP.mult
                )
                nc.vector.scalar_tensor_tensor(
                    yg[:], yg[:], -float(S_NORM), u2[:], op0=OP.mult, op1=OP.add
                )
                nc.scalar.dma_start(Od[s0:s1, 44:66, :], yg[:])

                # water (shortest chain, DVE+ACT):
                u = tp.tile([BLK, CW_CH, T], F32, tag="tmp")
                nc.scalar.activation(u[:], sw[:], AF.Square, bias=bias(6), scale=1.0)
                nc.vector.scalar_tensor_tensor(
                    u[:], u[:], aw[:], perm[:], op0=OP.mult, op1=OP.mult
                )
                nc.vector.scalar_tensor_tensor(
                    yw[:], yw[:], -float(S_NORM), u[:], op0=OP.mult, op1=OP.add
                )
                nc.scalar.dma_start(Od[s0:s1, 22:44, :], yw[:])

    _split_multi_waits(nc)
    _strip_init_barrier(nc)
    return nc


_NC_CACHE = None
LAST_RESULTS = None  # BassKernelResults of the most recent kernel() call


def _get_nc():
    global _NC_CACHE
    if _NC_CACHE is None:
        _NC_CACHE = _build()
    return _NC_CACHE


def kernel(X, Y):
    global LAST_RESULTS
    X = np.ascontiguousarray(np.asarray(X, dtype=np.float32))
    Y = np.ascontiguousarray(np.asarray(Y, dtype=np.float32))
    assert X.shape == (N_FULL, 89, T) and Y.shape == (N_FULL, 66, T)

    nc = _get_nc()
    carr = np.tile(np.array(_BIASES, np.float32)[None, :], (BLK, 1))
    in_maps = [
        {
            "X": X[i * S_CORE : (i + 1) * S_CORE],
            "Y": Y[i * S_CORE : (i + 1) * S_CORE],
            "C": carr,
        }
        for i in range(N_CORES)
    ]
    res = run_bass_kernel_spmd(nc, in_maps, core_ids=list(range(N_CORES)))
    LAST_RESULTS = res
    out = np.concatenate([r["O"] for r in res.results], axis=0)
    return out



# revision 6
# speedup vs baseline: 1.4320x; 1.4320x over previous
"""Trainium2 Bass kernel for the black-oil Peaceman loss (nn_Black_oil_peacemann).

Full inputs X:[4096,89,128] f32, Y:[4096,66,128] f32 -> out:[4096,66,128] f32.
Data-parallel over the batch axis: 512 samples per core on 8 cores; all math is
per-sample (the pressure mean is per-sample), the /N normalization uses the
global N=4096, so no cross-device communication is needed.

The kernel is DMA-bound, so all tensor I/O is fp16: the host converts the 67
used X channels (perm 0:22, pressure 22, Sg 45:67, Sw 67:89) and Y to fp16,
and the device returns a 2^-8-scaled fp16 output that the host upconverts.
The scaling is needed because the raw well rates q reach ~1.1e7 (fp16 max is
65504) while the final loss values ~1e-10 underflow fp16; the device computes
out' = (q - rate)*2^-8 (the 2^-8 folded into the Peaceman constants and Y
pre-scaled by 2^-8 on the host), and the host multiplies by 1e-10/4096*256.
This halves HBM traffic vs f32: (23+44+66)in + 66out channels * 512 * 128 * 2B
= 26.1 MB/core, ~72 us at the 360 GB/s 16-SDMA-engine aggregate.

Per-core layout: samples on the 128 SBUF partitions (4 blocks of 128 samples),
channels*T on the free axis.

Algebra (s = 1e-10/4096, K = 2*pi*DZ/ln(RE/RWELL), all C* include /256):
  p      = mean_t pressure;  dd = 100 - p;  m = min(p, 0.5)
  oil:   q = ((Sg-0.7)*(0.8-Sw))^2 * ao * perm,
         ao = CO*dd*exp(8e-5*m - 8e-6 - 1e-5*relu(p-0.5))
  water: q = (Sw-0.1)^2 * aw * perm,  aw = CW*dd
  gas:   q = Sg^2 * ag * perm,  ag = CG*dd/(mu_g(p)*bg(p))
  out'  = q - Y*2^-8   (host: out = out' * s*256)
DVE scalar_tensor_tensor gets no 16-bit speedup but tensor_tensor does (2x_1p),
so the per-sample factors are sqrt()ed and folded into the ACT-engine
Square(in*scale) ops ([128,1] scale/bias APs), leaving the perm-multiplies and
Y-subtracts as fp16 tensor_tensor ops at 2x DVE rate. Input DMAs ride the SP
ring, output DMAs the Pool ring, keeping ACT/DVE sequencers free.
"""

import math
import sys

if "/opt/trn_rl_repo" not in sys.path:
    sys.path.insert(0, "/opt/trn_rl_repo")

import numpy as np

import concourse.bass as bass
import concourse.mybir as mybir
import concourse.tile as tile
from concourse.bass_utils import run_bass_kernel_spmd
from concourse.vector_clock import ScopedClock

F32 = mybir.dt.float32
F16 = mybir.dt.float16
AF = mybir.ActivationFunctionType
OP = mybir.AluOpType

N_CORES = 8
N_FULL = 4096
S_CORE = N_FULL // N_CORES  # 512 samples per core
BLK = 128                   # samples per block == SBUF partitions
N_BLK = S_CORE // BLK       # 4
T = 128
CW_CH = 22                  # wells per phase

OUT_SCALE = 2.0 ** -8                         # device output = true_q * 2^-8
S_HOST = np.float32(1e-10 / N_FULL / OUT_SCALE)  # host multiplier on upconvert
RIGHT = float(np.log(np.float32(2.0)))        # ln(RE/RWELL), RE=400 RWELL=200
K_PEACE = 2.0 * math.pi * 100.0 / RIGHT       # 2*pi*DZ/right
C_W = float(np.float32(K_PEACE * (0.3 / 0.49) * OUT_SCALE))
# exp-bias constants folded multiplicatively (avoids non-zero ACT bias APs):
# bg = e^-1.7e-4 * Exp(1.7e-3*m)  ->  C_G *= e^1.7e-4 (bg is in the denominator)
# oil exp arg = (8*mn - r1)*1e-5/T - 8e-6  ->  C_O *= e^-8e-6
C_G = float(np.float32(K_PEACE * (0.8 / 0.49) * OUT_SCALE * math.exp(1.7e-4)))
C_O = float(
    np.float32(K_PEACE * (0.9 / 0.2401 / 2.5) * OUT_SCALE * math.exp(-8e-6))
)


def _patch_tile_drain():
    """walrus in this container rejects TPB_CTRL instructions carrying more
    than one sem wait ("Too many sync wait commands"); split the TileContext
    exit drain's waits into one-wait-per-instruction nops."""
    if getattr(tile.TileContext, "_drain_patched", False):
        return

    def _drain_and_barrier(self, tick_clock, wait_clock):
        nc = self.nc
        drain_inst = nc.sync.drain()
        wait_clock.add_sem_waits(
            drain_inst.ins, ScopedClock({None: tick_clock.global_clock})
        )
        si = drain_inst.ins.sync_info
        if si is not None and si.on_wait and len(si.on_wait) > 1:
            extra = list(si.on_wait[1:])
            del si.on_wait[1:]
            for w in extra:
                nop = nc.sync.nop(nofuse=True)
                nsi = nop.ins.sync_info
                if nsi is None:
                    nop.ins.sync_info = mybir.SyncInfo(on_wait=[w], on_update=[])
                else:
                    nsi.on_wait.append(w)

        nc.all_engine_barrier()
        assert self.sems is not None
        popped = nc._tile_sem_poison_stack.pop()
        assert popped is self._sem_poison
        nc.clear_and_free_semaphores(list(self.sems.allocated().values()))
        nc.all_engine_barrier()

    tile.TileContext._drain_and_barrier = _drain_and_barrier
    tile.TileContext._drain_patched = True


def _strip_init_barrier(nc):
    """Drop the Bass-init all-engine barrier (drain + EVSEM butterfly) from
    the entry block. Its EVSEM waits block every engine ~6.5us on runtime
    event-sem arming before the first DMA can issue. It only ordered the
    init const memsets (t~0.3us, Pool) against their first compute reader
    (t~14us) -- a margin of ~14us makes the barrier unnecessary, and the
    kernel-tail barrier still runs long after arming completes."""
    bb = nc.m.functions[0].blocks[0]
    bb.instructions = [
        ins
        for ins in bb.instructions
        if type(ins).__name__ not in ("InstDrain", "InstEventSemaphore")
    ]


def _split_multi_waits(nc):
    """This container's walrus encodes at most one sem wait per instruction
    ("Too many sync wait commands"); hoist extra waits onto engine-matched
    nops inserted immediately before the offending instruction."""
    import bass_rust

    n = 0
    for f in nc.m.functions:
        for bb in f.blocks:
            out = []
            for ins in bb.instructions:
                si = ins.sync_info
                if si is not None and si.on_wait and len(si.on_wait) > 1:
                    keep = si.on_wait[-1]
                    for w in list(si.on_wait[:-1]):
                        nop = bass_rust.InstNoOp(
                            name=f"I-waitsplit-{n}", ins=[], outs=[]
                        )
                        n += 1
                        nop.engine = ins.engine
                        nop.sync_info = mybir.SyncInfo(on_wait=[w], on_update=[])
                        nc.register_instruction(nop)
                        out.append(nop)
                    del si.on_wait[:]
                    si.on_wait.append(keep)
                out.append(ins)
            bb.instructions = out


def _build():
    _patch_tile_drain()
    nc = bass.Bass(trn_type="TRN2")
    XAd = nc.dram_tensor("XA", [S_CORE, 23, T], F16, kind="ExternalInput")
    XBd = nc.dram_tensor("XB", [S_CORE, 2 * CW_CH, T], F16, kind="ExternalInput")
    Yd = nc.dram_tensor("YS", [S_CORE, 66, T], F16, kind="ExternalInput")
    Od = nc.dram_tensor("O", [S_CORE, 66, T], F16, kind="ExternalOutput")

    with tile.TileContext(nc) as tc:
        with (
            tc.tile_pool(name="xa_p", bufs=3) as xap,
            tc.tile_pool(name="io", bufs=2) as iop,
            tc.tile_pool(name="tmp", bufs=4) as tp,
            tc.tile_pool(name="sc", bufs=2) as sp,
        ):
            for b in range(N_BLK):
                s0 = b * BLK
                s1 = s0 + BLK

                xa = xap.tile([BLK, 23, T], F16, tag="xa")
                nc.sync.dma_start(xa[:], XAd[s0:s1, :, :])
                xb = iop.tile([BLK, 2 * CW_CH, T], F16, tag="xb")
                nc.sync.dma_start(xb[:], XBd[s0:s1, :, :])
                y = iop.tile([BLK, 66, T], F16, tag="y")
                nc.sync.dma_start(y[:], Yd[s0:s1, :, :])
                perm = xa[:, 0:22, :]
                press = xa[:, 22:23, :]
                sg = xb[:, 0:22, :]
                sw = xb[:, 22:44, :]

                # ---- per-sample scalars ([128,1] f32); ps = 128*p ----
                ps = sp.tile([BLK, 1], F32, tag="ps")
                nc.vector.reduce_sum(ps[:], press[:], axis=mybir.AxisListType.X)
                dd = sp.tile([BLK, 1], F32, tag="dd")
                nc.vector.tensor_scalar(
                    dd[:], ps[:], -1.0 / T, 100.0, op0=OP.mult, op1=OP.add
                )
                mn = sp.tile([BLK, 1], F32, tag="mn")
                nc.vector.tensor_scalar_min(mn[:], ps[:], 0.5 * T)

                # oil factor ao = CO * dd * exp((8*mn - r1)*1e-5/T) (CO has e^-8e-6)
                r1 = sp.tile([BLK, 1], F32, tag="r1")
                nc.vector.tensor_scalar(
                    r1[:], ps[:], -0.5 * T, 0.0, op0=OP.add, op1=OP.max
                )
                u8 = sp.tile([BLK, 1], F32, tag="u8")
                nc.vector.scalar_tensor_tensor(
                    u8[:], mn[:], 8.0, r1[:], op0=OP.mult, op1=OP.subtract
                )
                ibo = sp.tile([BLK, 1], F32, tag="ibo")
                nc.scalar.activation(ibo[:], u8[:], AF.Exp, scale=1e-5 / T)
                ao = sp.tile([BLK, 1], F32, tag="ao")
                nc.vector.scalar_tensor_tensor(
                    ao[:], ibo[:], C_O, dd[:], op0=OP.mult, op1=OP.mult
                )
                sao = sp.tile([BLK, 1], F32, tag="sao")
                nc.scalar.activation(sao[:], ao[:], AF.Sqrt)

                # water factor aw = CW * dd; saw = sqrt(aw), bw = -0.1*saw
                saw = sp.tile([BLK, 1], F32, tag="saw")
                nc.scalar.activation(saw[:], dd[:], AF.Sqrt, scale=C_W)
                bw = sp.tile([BLK, 1], F32, tag="bw")
                nc.vector.tensor_scalar_mul(bw[:], saw[:], -0.1)

                # gas factor ag = CG * dd / (mu_g(p) * bg(p)); sag = sqrt(ag)
                sqp = sp.tile([BLK, 1], F32, tag="sqp")
                nc.scalar.activation(sqp[:], ps[:], AF.Square)
                pl = sp.tile([BLK, 1], F32, tag="pl")
                nc.vector.tensor_scalar(
                    pl[:], ps[:], 1e-6 / T, 0.0133, op0=OP.mult, op1=OP.add
                )
                mu = sp.tile([BLK, 1], F32, tag="mu")
                nc.vector.scalar_tensor_tensor(
                    mu[:], sqp[:], 3e-10 / (T * T), pl[:], op0=OP.mult, op1=OP.add
                )
                bgt = sp.tile([BLK, 1], F32, tag="bgt")
                nc.scalar.activation(bgt[:], mn[:], AF.Exp, scale=1.7e-3 / T)
                den = sp.tile([BLK, 1], F32, tag="den")
                nc.vector.tensor_mul(den[:], mu[:], bgt[:])
                rg = sp.tile([BLK, 1], F32, tag="rg")
                nc.vector.reciprocal(rg[:], den[:])
                ag = sp.tile([BLK, 1], F32, tag="ag")
                nc.vector.scalar_tensor_tensor(
                    ag[:], rg[:], C_G, dd[:], op0=OP.mult, op1=OP.mult
                )
                sag = sp.tile([BLK, 1], F32, tag="sag")
                nc.scalar.activation(sag[:], ag[:], AF.Sqrt)

                # ---- main elementwise over [128, 22, 128] fp16 ----
                yo = y[:, 0:22, :]
                yw = y[:, 22:44, :]
                yg = y[:, 44:66, :]

                # oil: q = Sq((sg-0.7)*(0.8-sw)*sao) * perm; yo <- q - yo
                v = tp.tile([BLK, CW_CH, T], F16, tag="tmp")
                nc.scalar.activation(v[:], sw[:], AF.Copy, bias=0.8, scale=-1.0)
                w = tp.tile([BLK, CW_CH, T], F16, tag="tmp")
                nc.vector.scalar_tensor_tensor(
                    w[:], sg[:], 0.7, v[:], op0=OP.subtract, op1=OP.mult
                )
                nc.scalar.activation(w[:], w[:], AF.Square, scale=sao[:])
                nc.vector.tensor_mul(w[:], w[:], perm[:])
                nc.vector.tensor_sub(yo[:], w[:], yo[:])
                nc.gpsimd.dma_start(Od[s0:s1, 0:22, :], yo[:])

                # gas: q = Sq(sg*sag) * perm; yg <- q - yg
                tg = tp.tile([BLK, CW_CH, T], F16, tag="tmp")
                nc.scalar.activation(tg[:], sg[:], AF.Square, scale=sag[:])
                nc.vector.tensor_mul(tg[:], tg[:], perm[:])
                nc.vector.tensor_sub(yg[:], tg[:], yg[:])
                nc.gpsimd.dma_start(Od[s0:s1, 44:66, :], yg[:])

                # water: q = Sq(sw*saw - 0.1*saw) * perm; yw <- q - yw
                tw = tp.tile([BLK, CW_CH, T], F16, tag="tmp")
                nc.scalar.activation(
                    tw[:], sw[:], AF.Square, bias=bw[:], scale=saw[:]
                )
                nc.vector.tensor_mul(tw[:], tw[:], perm[:])
                nc.vector.tensor_sub(yw[:], tw[:], yw[:])
                nc.gpsimd.dma_start(Od[s0:s1, 22:44, :], yw[:])

    _split_multi_waits(nc)
    _strip_init_barrier(nc)
    return nc


_NC_CACHE = None
LAST_RESULTS = None  # BassKernelResults of the most recent kernel() call


def _get_nc():
    global _NC_CACHE
    if _NC_CACHE is None:
        _NC_CACHE = _build()
    return _NC_CACHE


def kernel(X, Y):
    global LAST_RESULTS
    X = np.asarray(X)
    Y = np.asarray(Y)
    assert X.shape == (N_FULL, 89, T) and Y.shape == (N_FULL, 66, T)

    xa = np.ascontiguousarray(X[:, 0:23, :]).astype(np.float16)
    xb = np.ascontiguousarray(X[:, 45:89, :]).astype(np.float16)
    ys = (np.asarray(Y, dtype=np.float32) * np.float32(OUT_SCALE)).astype(
        np.float16
    )

    nc = _get_nc()
    in_maps = [
        {
            "XA": xa[i * S_CORE : (i + 1) * S_CORE],
            "XB": xb[i * S_CORE : (i + 1) * S_CORE],
            "YS": ys[i * S_CORE : (i + 1) * S_CORE],
        }
        for i in range(N_CORES)
    ]
    res = run_bass_kernel_spmd(nc, in_maps, core_ids=list(range(N_CORES)))
    LAST_RESULTS = res
    out = np.concatenate([r["O"] for r in res.results], axis=0)
    return out.astype(np.float32) * S_HOST


# revision 12
# speedup vs baseline: 1.4828x; 1.0355x over previous
"""Trainium2 Bass kernel for the black-oil Peaceman loss (nn_Black_oil_peacemann).

Full inputs X:[4096,89,128] f32, Y:[4096,66,128] f32 -> out:[4096,66,128] f32.
Data-parallel over the batch axis: 512 samples per core on 8 cores; all math is
per-sample (the pressure mean is per-sample), the /N normalization uses the
global N=4096, so no cross-device communication is needed.

The kernel is DMA-bound, so all tensor I/O is fp16: the host converts the 67
used X channels (perm 0:22, pressure 22, Sg 45:67, Sw 67:89) and Y to fp16,
and the device returns a 2^-8-scaled fp16 output that the host upconverts.
The scaling is needed because the raw well rates q reach ~1.1e7 (fp16 max is
65504) while the final loss values ~1e-10 underflow fp16; the device computes
out' = (q - rate)*2^-8 (the 2^-8 folded into the Peaceman constants and Y
pre-scaled by 2^-8 on the host), and the host multiplies by 1e-10/4096*256.
This halves HBM traffic vs f32: (23+44+66)in + 66out channels * 512 * 128 * 2B
= 26.1 MB/core, ~72 us at the 360 GB/s 16-SDMA-engine aggregate.

Per-core layout: samples on the 128 SBUF partitions (4 blocks of 128 samples),
channels*T on the free axis.

Algebra (s = 1e-10/4096, K = 2*pi*DZ/ln(RE/RWELL), all C* include /256):
  p      = mean_t pressure;  dd = 100 - p;  m = min(p, 0.5)
  oil:   q = ((Sg-0.7)*(0.8-Sw))^2 * ao * perm,
         ao = CO*dd*exp(8e-5*m - 8e-6 - 1e-5*relu(p-0.5))
  water: q = (Sw-0.1)^2 * aw * perm,  aw = CW*dd
  gas:   q = Sg^2 * ag * perm,  ag = CG*dd/(mu_g(p)*bg(p))
  out'  = q - Y*2^-8   (host: out = out' * s*256)
DVE scalar_tensor_tensor gets no 16-bit speedup but tensor_tensor does (2x_1p),
so the per-sample factors are sqrt()ed and folded into the ACT-engine
Square(in*scale) ops ([128,1] scale/bias APs), leaving the perm-multiplies and
Y-subtracts as fp16 tensor_tensor ops at 2x DVE rate. Input DMAs ride the SP
ring, output DMAs the Pool ring, keeping ACT/DVE sequencers free.
"""

import math
import sys

if "/opt/trn_rl_repo" not in sys.path:
    sys.path.insert(0, "/opt/trn_rl_repo")

import numpy as np

import concourse.bass as bass
import concourse.mybir as mybir
import concourse.tile as tile
from concourse.bass_utils import run_bass_kernel_spmd
from concourse.vector_clock import ScopedClock

F32 = mybir.dt.float32
F16 = mybir.dt.float16
AF = mybir.ActivationFunctionType
OP = mybir.AluOpType

N_CORES = 8
N_FULL = 4096
S_CORE = N_FULL // N_CORES  # 512 samples per core
BLK = 128                   # samples per block == SBUF partitions
N_BLK = S_CORE // BLK       # 4
T = 128
CW_CH = 22                  # wells per phase

OUT_SCALE = 2.0 ** -8                         # device output = true_q * 2^-8
S_HOST = np.float32(1e-10 / N_FULL / OUT_SCALE)  # host multiplier on upconvert
RIGHT = float(np.log(np.float32(2.0)))        # ln(RE/RWELL), RE=400 RWELL=200
K_PEACE = 2.0 * math.pi * 100.0 / RIGHT       # 2*pi*DZ/right
C_W = float(np.float32(K_PEACE * (0.3 / 0.49) * OUT_SCALE))
C_G = float(np.float32(K_PEACE * (0.8 / 0.49) * OUT_SCALE))
C_O = float(np.float32(K_PEACE * (0.9 / 0.2401 / 2.5) * OUT_SCALE))


def _patch_tile_drain():
    """walrus in this container rejects TPB_CTRL instructions carrying more
    than one sem wait ("Too many sync wait commands"); split the TileContext
    exit drain's waits into one-wait-per-instruction nops."""
    if getattr(tile.TileContext, "_drain_patched", False):
        return

    def _drain_and_barrier(self, tick_clock, wait_clock):
        nc = self.nc
        drain_inst = nc.sync.drain()
        wait_clock.add_sem_waits(
            drain_inst.ins, ScopedClock({None: tick_clock.global_clock})
        )
        si = drain_inst.ins.sync_info
        if si is not None and si.on_wait and len(si.on_wait) > 1:
            extra = list(si.on_wait[1:])
            del si.on_wait[1:]
            for w in extra:
                nop = nc.sync.nop(nofuse=True)
                nsi = nop.ins.sync_info
                if nsi is None:
                    nop.ins.sync_info = mybir.SyncInfo(on_wait=[w], on_update=[])
                else:
                    nsi.on_wait.append(w)

        nc.all_engine_barrier()
        assert self.sems is not None
        popped = nc._tile_sem_poison_stack.pop()
        assert popped is self._sem_poison
        nc.clear_and_free_semaphores(list(self.sems.allocated().values()))
        nc.all_engine_barrier()

    tile.TileContext._drain_and_barrier = _drain_and_barrier
    tile.TileContext._drain_patched = True


def _strip_init_barrier(nc):
    """Drop the Bass-init all-engine barrier (drain + EVSEM butterfly) from
    the entry block. Its EVSEM waits block every engine ~6.5us on runtime
    event-sem arming before the first DMA can issue. It only ordered the
    init const memsets (t~0.3us, Pool) against their first compute reader
    (t~14us) -- a margin of ~14us makes the barrier unnecessary, and the
    kernel-tail barrier still runs long after arming completes."""
    bb = nc.m.functions[0].blocks[0]
    bb.instructions = [
        ins
        for ins in bb.instructions
        if type(ins).__name__ not in ("InstDrain", "InstEventSemaphore")
    ]


def _split_multi_waits(nc):
    """This container's walrus encodes at most one sem wait per instruction
    ("Too many sync wait commands"); hoist extra waits onto engine-matched
    nops inserted immediately before the offending instruction."""
    import bass_rust

    n = 0
    for f in nc.m.functions:
        for bb in f.blocks:
            out = []
            for ins in bb.instructions:
                si = ins.sync_info
                if si is not None and si.on_wait and len(si.on_wait) > 1:
                    keep = si.on_wait[-1]
                    for w in list(si.on_wait[:-1]):
                        nop = bass_rust.InstNoOp(
                            name=f"I-waitsplit-{n}", ins=[], outs=[]
                        )
                        n += 1
                        nop.engine = ins.engine
                        nop.sync_info = mybir.SyncInfo(on_wait=[w], on_update=[])
                        nc.register_instruction(nop)
                        out.append(nop)
                    del si.on_wait[:]
                    si.on_wait.append(keep)
                out.append(ins)
            bb.instructions = out


def _build():
    _patch_tile_drain()
    nc = bass.Bass(trn_type="TRN2")
    XAd = nc.dram_tensor("XA", [S_CORE, 23, T], F16, kind="ExternalInput")
    XBd = nc.dram_tensor("XB", [S_CORE, 2 * CW_CH, T], F16, kind="ExternalInput")
    Yd = nc.dram_tensor("YS", [S_CORE, 66, T], F16, kind="ExternalInput")
    Od = nc.dram_tensor("O", [S_CORE, 66, T], F16, kind="ExternalOutput")

    with tile.TileContext(nc) as tc:
        with (
            tc.tile_pool(name="xa_p", bufs=3) as xap,
            tc.tile_pool(name="io", bufs=3) as iop,
            tc.tile_pool(name="tmp", bufs=4) as tp,
            tc.tile_pool(name="sc", bufs=2) as sp,
        ):
            for b in range(N_BLK):
                s0 = b * BLK
                s1 = s0 + BLK

                xa = xap.tile([BLK, 23, T], F16, tag="xa")
                nc.sync.dma_start(xa[:], XAd[s0:s1, :, :])
                xb = iop.tile([BLK, 2 * CW_CH, T], F16, tag="xb")
                nc.sync.dma_start(xb[:], XBd[s0:s1, :, :])
                y = iop.tile([BLK, 66, T], F16, tag="y")
                nc.sync.dma_start(y[:], Yd[s0:s1, :, :])
                perm = xa[:, 0:22, :]
                press = xa[:, 22:23, :]
                sg = xb[:, 0:22, :]
                sw = xb[:, 22:44, :]

                # ---- per-sample scalars ([128,1] f32); ps = 128*p ----
                ps = sp.tile([BLK, 1], F32, tag="ps")
                nc.vector.reduce_sum(ps[:], press[:], axis=mybir.AxisListType.X)
                dd = sp.tile([BLK, 1], F32, tag="dd")
                nc.vector.tensor_scalar(
                    dd[:], ps[:], -1.0 / T, 100.0, op0=OP.mult, op1=OP.add
                )
                mn = sp.tile([BLK, 1], F32, tag="mn")
                nc.vector.tensor_scalar_min(mn[:], ps[:], 0.5 * T)

                # oil factor ao = CO * dd * exp(e), e = (8*mn - r1)*1e-5/T - 8e-6.
                # |e| < 4e-5 so exp(e) = 1 + e to ~1e-9; no Exp needed (keeps
                # the ACT engine on the single sqrt_and_others act-table set).
                r1 = sp.tile([BLK, 1], F32, tag="r1")
                nc.vector.tensor_scalar(
                    r1[:], ps[:], -0.5 * T, 0.0, op0=OP.add, op1=OP.max
                )
                u8 = sp.tile([BLK, 1], F32, tag="u8")
                nc.vector.scalar_tensor_tensor(
                    u8[:], mn[:], 8.0, r1[:], op0=OP.mult, op1=OP.subtract
                )
                ibo = sp.tile([BLK, 1], F32, tag="ibo")
                nc.vector.tensor_scalar(
                    ibo[:], u8[:], 1e-5 / T, 1.0 - 8e-6, op0=OP.mult, op1=OP.add
                )
                ao = sp.tile([BLK, 1], F32, tag="ao")
                nc.vector.scalar_tensor_tensor(
                    ao[:], ibo[:], C_O, dd[:], op0=OP.mult, op1=OP.mult
                )
                sao = sp.tile([BLK, 1], F32, tag="sao")
                nc.scalar.activation(sao[:], ao[:], AF.Sqrt)

                # water factor aw = CW * dd; saw = sqrt(aw), bw = -0.1*saw
                saw = sp.tile([BLK, 1], F32, tag="saw")
                nc.scalar.activation(saw[:], dd[:], AF.Sqrt, scale=C_W)
                bw = sp.tile([BLK, 1], F32, tag="bw")
                nc.vector.tensor_scalar_mul(bw[:], saw[:], -0.1)

                # gas factor ag = CG * dd / (mu_g(p) * bg(p)); sag = sqrt(ag)
                sqp = sp.tile([BLK, 1], F32, tag="sqp")
                nc.vector.tensor_mul(sqp[:], ps[:], ps[:])
                pl = sp.tile([BLK, 1], F32, tag="pl")
                nc.vector.tensor_scalar(
                    pl[:], ps[:], 1e-6 / T, 0.0133, op0=OP.mult, op1=OP.add
                )
                mu = sp.tile([BLK, 1], F32, tag="mu")
                nc.vector.scalar_tensor_tensor(
                    mu[:], sqp[:], 3e-10 / (T * T), pl[:], op0=OP.mult, op1=OP.add
                )
                # bg = exp(x), x = 1.7e-3*m - 1.7e-4, |x| < 7e-4 -> 1 + x
                bgt = sp.tile([BLK, 1], F32, tag="bgt")
                nc.vector.tensor_scalar(
                    bgt[:], mn[:], 1.7e-3 / T, 1.0 - 1.7e-4, op0=OP.mult, op1=OP.add
                )
                den = sp.tile([BLK, 1], F32, tag="den")
                nc.vector.tensor_mul(den[:], mu[:], bgt[:])
                rg = sp.tile([BLK, 1], F32, tag="rg")
                nc.vector.reciprocal(rg[:], den[:])
                ag = sp.tile([BLK, 1], F32, tag="ag")
                nc.vector.scalar_tensor_tensor(
                    ag[:], rg[:], C_G, dd[:], op0=OP.mult, op1=OP.mult
                )
                sag = sp.tile([BLK, 1], F32, tag="sag")
                nc.scalar.activation(sag[:], ag[:], AF.Sqrt)

                # ---- main elementwise over [128, 22, 128] fp16 ----
                yo = y[:, 0:22, :]
                yw = y[:, 22:44, :]
                yg = y[:, 44:66, :]

                # oil: q = Sq((sg-0.7)*(0.8-sw)*sao) * perm; yo <- q - yo
                v = tp.tile([BLK, CW_CH, T], F16, tag="tmp")
                nc.scalar.activation(v[:], sw[:], AF.Copy, bias=0.8, scale=-1.0)
                w = tp.tile([BLK, CW_CH, T], F16, tag="tmp")
                nc.vector.scalar_tensor_tensor(
                    w[:], sg[:], 0.7, v[:], op0=OP.subtract, op1=OP.mult
                )
                nc.scalar.activation(w[:], w[:], AF.Square, scale=sao[:])
                nc.vector.tensor_mul(w[:], w[:], perm[:])
                nc.vector.tensor_sub(yo[:], w[:], yo[:])
                nc.gpsimd.dma_start(Od[s0:s1, 0:22, :], yo[:])

                # gas: q = Sq(sg*sag) * perm; yg <- q - yg
                tg = tp.tile([BLK, CW_CH, T], F16, tag="tmp")
                nc.scalar.activation(tg[:], sg[:], AF.Square, scale=sag[:])
                nc.vector.tensor_mul(tg[:], tg[:], perm[:])
                nc.vector.tensor_sub(yg[:], tg[:], yg[:])
                nc.gpsimd.dma_start(Od[s0:s1, 44:66, :], yg[:])

                # water: q = Sq(sw*saw - 0.1*saw) * perm; yw <- q - yw
                tw = tp.tile([BLK, CW_CH, T], F16, tag="tmp")
                nc.scalar.activation(
                    tw[:], sw[:], AF.Square, bias=bw[:], scale=saw[:]
                )
                nc.vector.tensor_mul(tw[:], tw[:], perm[:])
                nc.vector.tensor_sub(yw[:], tw[:], yw[:])
                nc.gpsimd.dma_start(Od[s0:s1, 22:44, :], yw[:])

    _split_multi_waits(nc)
    _strip_init_barrier(nc)
    return nc


_NC_CACHE = None
LAST_RESULTS = None  # BassKernelResults of the most recent kernel() call


def _get_nc():
    global _NC_CACHE
    if _NC_CACHE is None:
        _NC_CACHE = _build()
    return _NC_CACHE


def kernel(X, Y):
    global LAST_RESULTS
    X = np.asarray(X)
    Y = np.asarray(Y)
    assert X.shape == (N_FULL, 89, T) and Y.shape == (N_FULL, 66, T)

    xa = np.ascontiguousarray(X[:, 0:23, :]).astype(np.float16)
    xb = np.ascontiguousarray(X[:, 45:89, :]).astype(np.float16)
    ys = (np.asarray(Y, dtype=np.float32) * np.float32(OUT_SCALE)).astype(
        np.float16
    )

    nc = _get_nc()
    in_maps = [
        {
            "XA": xa[i * S_CORE : (i + 1) * S_CORE],
            "XB": xb[i * S_CORE : (i + 1) * S_CORE],
            "YS": ys[i * S_CORE : (i + 1) * S_CORE],
        }
        for i in range(N_CORES)
    ]
    res = run_bass_kernel_spmd(nc, in_maps, core_ids=list(range(N_CORES)))
    LAST_RESULTS = res
    out = np.concatenate([r["O"] for r in res.results], axis=0)
    return out.astype(np.float32) * S_HOST
